# revision 16
# baseline (speedup 1.0000x reference)
"""CPCC loss (1 - Pearson(tree_d, proto_d)) on 8 Trainium2 NeuronCores.

Strategy (data-parallel, per sharding hint):
  - Shard representations/target_fine along N across the 8 cores (contiguous
    32768-row blocks).
  - Each core streams its 16 MiB of representations from HBM (SWDGE DMA with
    inline f32->bf16 cast; fully contiguous reads AND writes per partition,
    16 x 1 MiB tiles). For every 128-row chunk a bf16 one-hot
    [128 tokens x 128 classes(padded)] is built on DVE (is_equal against an
    iota constant, 16 chunks per instruction) and one PE matmul per chunk
    accumulates  onehot.T @ reps  into a [128, 128] f32 PSUM tile -> per-core
    segment sums. The pad classes never match so their rows stay zero.
    Measured: the stream runs at the effective HBM ceiling (~330 GB/s/core
    with all 8 cores streaming); one-hot+matmul fully hide under the DMA.
  - Segment COUNTS depend only on target_fine, so 1/max(count,1) is
    precomputed on the host (like the hierarchy selector matrices) and fused
    into the PSUM->SBUF copy on the scalar engine (Copy activation with a
    per-partition scale AP); the partials are therefore pre-averaged and an
    AllReduce(add) of the [100, 128] partials yields the fine prototypes
    directly (measured cheaper than AllGather + local tree-sum).
  - Every core then runs the tiny replicated tail: mid+coarse prototypes in
    ONE matmul (host-packed [wm | wm@wc]), both transposed into one packed
    [D, 125] tile, two Gram->distance blocks (fine 100x100 and mid+coarse
    25x25; cross distances never needed; d2 clamped at 0 before sqrt),
    tree-distance expansion T = B^T (D25*blockmask) B with a host-packed
    selector B=[Em;Ec], Pearson sums via DVE free-axis accumulators (the
    two P-only sums start before T is ready) + one PE column-sum matmul,
    and a short scalar epilogue (fast-approx reciprocal). Off-diagonal
    masking is skipped: diagonal terms are O(sqrt(EPS)) = 1e-6, negligible
    against sums of ~1e3 in f32, and the NPAIRS-corrected moment formulas
    only count true pairs.

Precision: only the representations are rounded to bf16 (matmul operand);
accumulation is f32 in PSUM and the whole tail is f32. Observed loss error
vs the f32 reference is ~1e-4 relative.
"""

import numpy as np

C_FINE, C_MID, C_COARSE = 100, 20, 5
EPS = 1e-12
N_CORES = 8
N, D = 262144, 128
N_LOC = N // N_CORES            # 32768 rows per core
CHUNK = 128                     # contraction size per matmul
N_CHUNKS = N_LOC // CHUNK       # 256
TILE_CHUNKS = 16                # chunks per DMA tile (2048 rows = 1 MiB)
N_TILES = N_CHUNKS // TILE_CHUNKS
OH_BATCH = 16                   # one-hot chunks built per DVE op
NPAIRS = C_FINE * (C_FINE - 1) // 2   # 4950
NP_CK = 25                      # packed mid+coarse rows (20 + 5)
NP_ALL = C_FINE + NP_CK         # 125 packed prototype rows

_CACHE = {}


def _build_program(stream_reps=1, loop_reps=1, dma_only=False, no_cc=False,
                   cc_tail_reps=1, cc_kind="ar", skip_tail=False,
                   tail_cut=None, no_dma=False, tile_chunks=TILE_CHUNKS,
                   dual_queue=False):
    """Build the SPMD program.

    Benchmarking knobs (the graded kernel uses all defaults):
      stream_reps>1 statically unrolls the streaming phase (same data).
      loop_reps>1 wraps the streaming phase in a dynamic For_i loop (slope
        timing); psum restarts each rep so the output stays correct.
      dma_only=True keeps only 1 one-hot + 1 matmul per tile.
      no_cc=True builds a single-core program without the collective.
      cc_tail_reps>1 serially chains the collective+tail section that many
        times (slope timing of the non-streaming part; output garbage).
      cc_kind: "ar" AllReduce(add) (default); "ag" AllGather + local sum.
      skip_tail=True: loss <- prot[0,0] right after the collective+sum.
    """
    import contextlib

    import concourse.bacc as bacc
    import concourse.mybir as mybir
    import concourse.tile as tile
    from concourse.bass import MemorySpace

    f32 = mybir.dt.float32
    bf16 = mybir.dt.bfloat16
    Alu = mybir.AluOpType
    Act = mybir.ActivationFunctionType
    X = mybir.AxisListType.X

    nc = bacc.Bacc("TRN2", target_bir_lowering=False, debug=False,
                   num_devices=1 if no_cc else N_CORES)

    reps_d = nc.dram_tensor("reps", [N_LOC, D], f32, kind="ExternalInput")
    tgtT_d = nc.dram_tensor("tgtT", [CHUNK, N_CHUNKS], bf16,
                            kind="ExternalInput")
    iota_d = nc.dram_tensor("iota", [128, OH_BATCH * CHUNK], bf16,
                            kind="ExternalInput")
    ident_d = nc.dram_tensor("ident", [128, 128], f32, kind="ExternalInput")
    ones_d = nc.dram_tensor("ones", [128, 128], f32, kind="ExternalInput")
    invc_d = nc.dram_tensor("invc", [C_FINE, 1], f32, kind="ExternalInput")
    wpk_d = nc.dram_tensor("wpk", [C_FINE, NP_CK], f32, kind="ExternalInput")
    bpk_d = nc.dram_tensor("bpk", [NP_CK, C_FINE], f32, kind="ExternalInput")
    msk_d = nc.dram_tensor("msk", [NP_CK, NP_CK], f32, kind="ExternalInput")
    loss_d = nc.dram_tensor("loss", [1, 1], f32, kind="ExternalOutput")

    with tile.TileContext(nc) as tc:
        with (
            tc.tile_pool(name="const", bufs=1) as cpool,
            tc.tile_pool(name="reps", bufs=6) as rpool,
            tc.tile_pool(name="oh", bufs=3) as opool,
            tc.tile_pool(name="work", bufs=1) as wpool,
            tc.tile_pool(name="acc", bufs=1, space=MemorySpace.PSUM) as apool,
            tc.tile_pool(name="tps", bufs=4, space=MemorySpace.PSUM) as ppool,
            tc.tile_pool(name="dram", bufs=1, space=MemorySpace.DRAM) as dpool,
        ):
            # ---- constants (target first: it gates the whole DVE chain) ----
            tgtf_t = cpool.tile([CHUNK, N_CHUNKS], bf16)
            nc.sync.dma_start(tgtf_t[:], tgtT_d[:])
            iota_t = cpool.tile([128, OH_BATCH, CHUNK], bf16)
            nc.sync.dma_start(
                iota_t[:],
                iota_d[:].rearrange("p (g c) -> p g c", c=CHUNK))

            ident_t = cpool.tile([128, 128], f32)
            nc.sync.dma_start(ident_t[:], ident_d[:])
            ones_t = cpool.tile([128, 128], f32)
            nc.sync.dma_start(ones_t[:], ones_d[:])
            invc_t = cpool.tile([C_FINE, 1], f32)
            nc.sync.dma_start(invc_t[:], invc_d[:])
            wpk_t = cpool.tile([C_FINE, NP_CK], f32)
            nc.sync.dma_start(wpk_t[:], wpk_d[:])
            bpk_t = cpool.tile([NP_CK, C_FINE], f32)
            nc.sync.dma_start(bpk_t[:], bpk_d[:])
            msk_t = cpool.tile([NP_CK, NP_CK], f32)
            nc.sync.dma_start(msk_t[:], msk_d[:])
            eps5_t = cpool.tile([128, 1], f32)
            nc.vector.memset(eps5_t[:], 1e-5)
            eps12_t = cpool.tile([128, 1], f32)
            nc.vector.memset(eps12_t[:], EPS)

            # ---- main streaming loop: segment sums ----
            n_tiles = N_CHUNKS // tile_chunks
            acc = apool.tile([CHUNK, D], f32)
            loop_cm = (tc.For_i(0, loop_reps, 1) if loop_reps > 1
                       else contextlib.nullcontext())
            with loop_cm:
                for rep in range(stream_reps):
                    for t in range(n_tiles):
                        rt = rpool.tile([128, tile_chunks, D], bf16, tag="rt")
                        src = reps_d[t * tile_chunks * CHUNK:
                                     (t + 1) * tile_chunks * CHUNK, :]
                        # row = p*tile_chunks + k -> each partition reads one
                        # fully contiguous block from HBM and writes one
                        # fully contiguous bf16 block; SWDGE casts inline
                        eng = (nc.sync if (dual_queue and t % 2 == 1)
                               else nc.gpsimd)
                        if not no_dma:
                            eng.dma_start(
                                rt[:],
                                src.rearrange("(p k) d -> p k d",
                                              k=tile_chunks)
                            )
                        n_b = 1 if dma_only else tile_chunks // OH_BATCH
                        for b in range(n_b):
                            oh = opool.tile([128, OH_BATCH, CHUNK], bf16,
                                            tag="oh")
                            c0 = t * tile_chunks + b * OH_BATCH
                            tgt_b = (tgtf_t[:, c0:c0 + OH_BATCH]
                                     .rearrange("p (g o) -> p g o", o=1)
                                     .broadcast_to([128, OH_BATCH, CHUNK]))
                            nc.vector.tensor_tensor(
                                oh[:], iota_t[:], tgt_b, Alu.is_equal)
                            js = [0] if dma_only else range(OH_BATCH)
                            for j in js:
                                k = b * OH_BATCH + j
                                nc.tensor.matmul(
                                    acc[:], oh[:, j, :], rt[:, k, :],
                                    start=(rep == 0 and t == 0 and k == 0),
                                    stop=(rep == stream_reps - 1
                                          and t == n_tiles - 1
                                          and (k == tile_chunks - 1
                                               or dma_only)),
                                )

            # pre-scale partials by 1/max(count,1): prototypes sum linearly
            # across cores, so the scale can be applied before the collective
            part_t = wpool.tile([C_FINE, D], f32)
            nc.scalar.activation(part_t[:], acc[0:C_FINE, :], Act.Copy,
                                 scale=invc_t[:])

            if not no_cc:
                cc_in = dpool.tile([C_FINE, D], f32)
                if cc_kind == "ag":
                    cc_out = dpool.tile([N_CORES * C_FINE, D], f32)
                else:
                    cc_out = dpool.tile([C_FINE, D], f32)

            def gather_summed(prot):
                """Collective the per-core partials; fine protos -> prot[0:100]."""
                if no_cc:
                    nc.vector.tensor_copy(prot[0:C_FINE, :], part_t[:])
                    return
                nc.sync.dma_start(cc_in[:], part_t[:])
                if cc_kind == "ag":
                    nc.gpsimd.collective_compute(
                        "AllGather",
                        mybir.AluOpType.bypass,
                        replica_groups=[list(range(N_CORES))],
                        ins=[cc_in.opt()],
                        outs=[cc_out.opt()],
                    )
                    gath = wpool.tile([C_FINE, N_CORES, D], f32, tag="gath")
                    nc.sync.dma_start(
                        gath[:],
                        cc_out[:].rearrange("(r c) f -> c r f", r=N_CORES))
                    nc.vector.tensor_add(gath[:, 0:4, :], gath[:, 0:4, :],
                                         gath[:, 4:8, :])
                    nc.vector.tensor_add(gath[:, 0:2, :], gath[:, 0:2, :],
                                         gath[:, 2:4, :])
                    nc.vector.tensor_add(prot[0:C_FINE, :], gath[:, 0, :],
                                         gath[:, 1, :])
                else:
                    nc.gpsimd.collective_compute(
                        "AllReduce",
                        mybir.AluOpType.add,
                        replica_groups=[list(range(N_CORES))],
                        ins=[cc_in.opt()],
                        outs=[cc_out.opt()],
                    )
                    nc.sync.dma_start(prot[0:C_FINE, :], cc_out[:])

            def cut_ls(src):
                ls = wpool.tile([1, 1], f32, tag="cut")
                nc.vector.tensor_copy(ls[:], src)
                return ls

            def tail(prot):
                """prot[0:100] = fine prototypes (already averaged).
                Returns the [1,1] loss tile."""
                # mid+coarse prototypes in one matmul: [wm | wm@wc]^T @ P
                ps_mc = ppool.tile([NP_CK, D], f32, tag="tps")
                nc.tensor.matmul(ps_mc[:], wpk_t[:], prot[0:C_FINE, :],
                                 start=True, stop=True)
                mc = wpool.tile([NP_CK, D], f32)
                nc.vector.tensor_copy(mc[:], ps_mc[:])
                if tail_cut == 1:
                    return cut_ls(mc[0:1, 0:1])

                # transpose all 125 prototype rows into one [D, 125] tile
                # (partition starts must be quadrant-aligned, so P and MC
                # stay in separate partition-0 tiles and pack along free)
                ps_tr = ppool.tile([D, NP_ALL], f32, tag="tps")
                nc.tensor.transpose(ps_tr[:, 0:C_FINE], prot[0:C_FINE, :],
                                    ident_t[0:C_FINE, 0:C_FINE])
                nc.tensor.transpose(ps_tr[:, C_FINE:NP_ALL], mc[:],
                                    ident_t[0:NP_CK, 0:NP_CK])
                XT = wpool.tile([D, NP_ALL], f32)
                nc.vector.tensor_copy(XT[:], ps_tr[:])
                if tail_cut == 2:
                    return cut_ls(XT[0:1, 0:1])

                # Gram -> distance matrices, one 100x100 block for the fine
                # protos and one 25x25 block for mid+coarse (cross distances
                # are never needed)
                # d2[i,j] = n_i + n_j - 2 G[i,j];  psum = G - (n_i + n_j)/2
                # dist = sqrt(max(-2*psum, 0) + EPS)
                x2 = wpool.tile([D, NP_ALL], f32)
                nc.vector.tensor_mul(x2[:], XT[:], XT[:])
                ps_n = ppool.tile([1, NP_ALL], f32, tag="tps")
                nc.tensor.matmul(ps_n[:], ones_t[:, 0:1], x2[:],
                                 start=True, stop=True)
                nm = wpool.tile([1, NP_ALL], f32)
                nc.vector.tensor_scalar(nm[:], ps_n[:], -0.5, None, Alu.mult)

                ps_g = ppool.tile([C_FINE, C_FINE], f32, tag="tps")
                nc.tensor.matmul(ps_g[:], XT[:, 0:C_FINE], XT[:, 0:C_FINE],
                                 start=True, stop=False)
                nc.tensor.matmul(ps_g[:], ones_t[0:1, 0:C_FINE],
                                 nm[:, 0:C_FINE], start=False, stop=False)
                nc.tensor.matmul(ps_g[:], nm[:, 0:C_FINE],
                                 ones_t[0:1, 0:C_FINE],
                                 start=False, stop=True)
                d2 = wpool.tile([C_FINE, C_FINE], f32)
                nc.vector.tensor_scalar(d2[:], ps_g[:], -2.0, 0.0,
                                        Alu.mult, Alu.max)
                Dfv = wpool.tile([C_FINE, C_FINE], f32)
                nc.scalar.activation(Dfv[:], d2[:], Act.Sqrt,
                                     bias=eps12_t[0:C_FINE, 0:1], scale=1.0)
                Df = Dfv[:]
                if tail_cut == 3:
                    return cut_ls(Dfv[0:1, 0:1])

                ps_q = ppool.tile([NP_CK, NP_CK], f32, tag="tps")
                nc.tensor.matmul(ps_q[:], XT[:, C_FINE:NP_ALL],
                                 XT[:, C_FINE:NP_ALL],
                                 start=True, stop=False)
                nc.tensor.matmul(ps_q[:], ones_t[0:1, 0:NP_CK],
                                 nm[:, C_FINE:NP_ALL],
                                 start=False, stop=False)
                nc.tensor.matmul(ps_q[:], nm[:, C_FINE:NP_ALL],
                                 ones_t[0:1, 0:NP_CK],
                                 start=False, stop=True)
                d2q = wpool.tile([NP_CK, NP_CK], f32)
                nc.vector.tensor_scalar(d2q[:], ps_q[:], -2.0, 0.0,
                                        Alu.mult, Alu.max)
                DIq = wpool.tile([NP_CK, NP_CK], f32)
                nc.scalar.activation(DIq[:], d2q[:], Act.Sqrt,
                                     bias=eps12_t[0:NP_CK, 0:1], scale=1.0)
                if tail_cut == 4:
                    return cut_ls(DIq[0:1, 0:1])

                # Pearson sums. Diagonal contributions are O(sqrt(EPS))=1e-6
                # vs sums ~1e3 — negligible in f32, so no masking needed.
                # red cols: 0=sum(T) 1=sum(P) 2=sum(T*P) 3=sum(T^2) 4=sum(P^2)
                # 1 and 4 only need Df — off T's critical path.
                red = wpool.tile([C_FINE, 8], f32, tag="red")
                nc.vector.reduce_sum(red[:, 1:2], Df, axis=X)
                pp_s = wpool.tile([C_FINE, C_FINE], f32, tag="pp")
                nc.vector.scalar_tensor_tensor(
                    pp_s[:], Df, 1.0, Df, Alu.mult, Alu.mult,
                    accum_out=red[:, 4:5])

                # tree distance expansion: T = B^T (DQ*mask) B over fine pairs
                dq = wpool.tile([NP_CK, NP_CK], f32)
                nc.vector.tensor_mul(dq[:], DIq[:], msk_t[:])
                ps_y = ppool.tile([NP_CK, C_FINE], f32, tag="tps")
                nc.tensor.matmul(ps_y[:], dq[:], bpk_t[:],
                                 start=True, stop=True)
                Y = wpool.tile([NP_CK, C_FINE], f32)
                nc.vector.tensor_copy(Y[:], ps_y[:])
                ps_T = ppool.tile([C_FINE, C_FINE], f32, tag="tps")
                nc.tensor.matmul(ps_T[:], bpk_t[:], Y[:],
                                 start=True, stop=True)

                tp_s = wpool.tile([C_FINE, C_FINE], f32, tag="tp")
                nc.vector.scalar_tensor_tensor(
                    tp_s[:], ps_T[:], 1.0, Df, Alu.mult, Alu.mult,
                    accum_out=red[:, 2:3])
                Tsb = wpool.tile([C_FINE, C_FINE], f32, tag="Tsb")
                nc.vector.tensor_scalar(
                    Tsb[:], ps_T[:], 1.0, 0.0, Alu.mult, Alu.add,
                    accum_out=red[:, 0:1])
                tt_s = wpool.tile([C_FINE, C_FINE], f32, tag="tt")
                nc.vector.scalar_tensor_tensor(
                    tt_s[:], Tsb[:], 1.0, Tsb[:], Alu.mult, Alu.mult,
                    accum_out=red[:, 3:4])

                ps_red = ppool.tile([1, 5], f32, tag="tps")
                nc.tensor.matmul(ps_red[:], ones_t[0:C_FINE, 0:1],
                                 red[:, 0:5], start=True, stop=True)
                f_s = ps_red
                if tail_cut == 5:
                    return cut_ls(f_s[0:1, 0:1])

                # num = F3/2 - F1*F2/19800 ; dt = F4/2 - F1^2/19800
                # dp = F5/2 - F2^2/19800 ; loss = 1 - num/sqrt(dt*dp + EPS)
                inv = 1.0 / (4.0 * NPAIRS)
                h = wpool.tile([1, 2], f32)
                nc.vector.tensor_scalar(h[:], f_s[:, 0:2], inv, None,
                                        Alu.mult)
                p2 = wpool.tile([1, 2], f32)
                nc.vector.tensor_mul(p2[:], h[:], f_s[:, 0:2])
                ab = wpool.tile([1, 1], f32)
                nc.vector.tensor_mul(ab[:], h[:, 0:1], f_s[:, 1:2])
                num = wpool.tile([1, 1], f32)
                nc.vector.scalar_tensor_tensor(
                    num[:], f_s[:, 2:3], 0.5, ab[:], Alu.mult, Alu.subtract)
                dd = wpool.tile([1, 2], f32)
                nc.vector.scalar_tensor_tensor(
                    dd[:], f_s[:, 3:5], 0.5, p2[:], Alu.mult, Alu.subtract)
                den = wpool.tile([1, 1], f32)
                nc.vector.tensor_mul(den[:], dd[:, 0:1], dd[:, 1:2])
                sq = wpool.tile([1, 1], f32)
                nc.scalar.activation(sq[:], den[:], Act.Sqrt,
                                     bias=eps12_t[0:1, 0:1], scale=1.0)
                rsq = wpool.tile([1, 1], f32)
                nc.vector.reciprocal_approx_fast(rsq[:], sq[:])
                ncorr = wpool.tile([1, 1], f32)
                nc.vector.scalar_tensor_tensor(
                    ncorr[:], num[:], -1.0, rsq[:], Alu.mult, Alu.mult)
                loss_t = wpool.tile([1, 1], f32)
                nc.vector.tensor_scalar(loss_t[:], ncorr[:], 1.0, None,
                                        Alu.add)
                return loss_t

            def round_(prot):
                gather_summed(prot)
                if skip_tail:
                    ls = wpool.tile([1, 1], f32)
                    nc.vector.tensor_copy(ls[:], prot[0:1, 0:1])
                    return ls
                return tail(prot)

            prot = wpool.tile([C_FINE, D], f32, tag="prot")
            prev = round_(prot)
            for _ in range(cc_tail_reps - 1):
                # serial chain: poke the previous loss into the partials so
                # the next collective+tail cannot start before it (bench only)
                nc.vector.tensor_copy(part_t[0:1, 0:1], prev[:])
                prot2 = wpool.tile([C_FINE, D], f32, tag="prot")
                prev = round_(prot2)
            nc.sync.dma_start(loss_d[:], prev[:])

    nc.compile()
    return nc


def _host_tgtT(tgt_loc, tile_chunks=TILE_CHUNKS):
    """Per-core target layout matching the device DMA: chunk (t, k) holds
    tokens {t*tile_chunks*128 + p*tile_chunks + k}, so
    tgtT[p, t*TC + k] = tgt[t*TC*128 + p*TC + k]."""
    import ml_dtypes

    n_tiles = N_CHUNKS // tile_chunks
    return np.ascontiguousarray(
        tgt_loc.reshape(n_tiles, 128, tile_chunks)
        .transpose(1, 0, 2).reshape(128, N_CHUNKS)
        .astype(np.float32)).astype(ml_dtypes.bfloat16)


def _host_constants(target_fine, fine2mid, fine2coarse):
    import ml_dtypes

    f2m = np.asarray(fine2mid, dtype=np.int64)
    f2c = np.asarray(fine2coarse, dtype=np.int64)
    iota = np.ascontiguousarray(np.broadcast_to(
        np.arange(CHUNK, dtype=np.float32),
        (128, OH_BATCH, CHUNK))).reshape(
            128, OH_BATCH * CHUNK).astype(ml_dtypes.bfloat16)
    ident = np.eye(128, dtype=np.float32)
    ones = np.ones((128, 128), dtype=np.float32)
    # global fine-class counts -> 1/max(cnt,1) (device partials sum linearly)
    cnt = np.bincount(np.asarray(target_fine, dtype=np.int64),
                      minlength=C_FINE).astype(np.float32)
    invc = (1.0 / np.maximum(cnt, 1.0)).reshape(C_FINE, 1).astype(np.float32)
    # selector / averaging matrices from the actual hierarchy inputs
    emt = (f2m[None, :] == np.arange(C_MID)[:, None]).astype(np.float32)
    cnt_m = np.maximum(np.bincount(f2m, minlength=C_MID), 1).astype(np.float32)
    wm = (emt / cnt_m[:, None]).T.astype(np.float32)     # [C_FINE, C_MID]
    # mid2coarse[m] = segment_max of fine2coarse over fines with fine2mid==m
    m2c = np.full(C_MID, -(2**31), dtype=np.int64)
    np.maximum.at(m2c, f2m, f2c)
    emc = (m2c[None, :] == np.arange(C_COARSE)[:, None]).astype(np.float32)
    cnt_c = np.maximum(emc.sum(axis=1), 1).astype(np.float32)
    wc = (emc / cnt_c[:, None]).T.astype(np.float32)     # [C_MID, C_COARSE]
    ect_sel = (f2c[None, :] == np.arange(C_COARSE)[:, None]).astype(np.float32)
    # packed: mid+coarse protos from fine protos in one matmul
    wpk = np.concatenate([wm, wm @ wc], axis=1)          # [C_FINE, 25]
    # packed selector [Em; Ec] and its block-diag mask
    bpk = np.concatenate([emt, ect_sel], axis=0)         # [25, C_FINE]
    msk = np.zeros((NP_CK, NP_CK), dtype=np.float32)
    msk[0:C_MID, 0:C_MID] = 1.0
    msk[C_MID:NP_CK, C_MID:NP_CK] = 1.0
    return {
        "iota": iota, "ident": ident, "ones": ones, "invc": invc,
        "wpk": np.ascontiguousarray(wpk),
        "bpk": np.ascontiguousarray(bpk),
        "msk": msk,
    }


def _make_in_maps(representations, target_fine, fine2mid, fine2coarse,
                  tile_chunks=TILE_CHUNKS):
    reps = np.ascontiguousarray(np.asarray(representations, dtype=np.float32))
    tgt = np.asarray(target_fine, dtype=np.int32)
    consts = _host_constants(tgt, fine2mid, fine2coarse)
    in_maps = []
    for r in range(N_CORES):
        lo, hi = r * N_LOC, (r + 1) * N_LOC
        in_maps.append({
            "reps": reps[lo:hi],
            "tgtT": _host_tgtT(tgt[lo:hi], tile_chunks),
            **consts,
        })
    return in_maps


def kernel(representations, target_fine, fine2mid, fine2coarse):
    from concourse.bass_utils import run_bass_kernel_spmd

    assert np.asarray(representations).shape == (N, D)
    assert np.asarray(target_fine).shape == (N,)

    if "nc" not in _CACHE:
        _CACHE["nc"] = _build_program()
    nc = _CACHE["nc"]

    in_maps = _make_in_maps(representations, target_fine,
                            fine2mid, fine2coarse)
    res = run_bass_kernel_spmd(nc, in_maps, core_ids=list(range(N_CORES)))
    loss = res.results[0]["loss"][0, 0]
    return np.asarray(loss, dtype=np.float32).reshape(())
